# revision 8
# baseline (speedup 1.0000x reference)
"""Cross-attention processor (nn_CaptureCrossAttentionProcessor) on 8 TRN2 cores.

Math (per batch element b):
    x  = hidden_states[b].reshape(C, N).T            # (N, C), N = 64*64
    q  = x @ Wq ; k = ehs[b] @ Wk ; v = ehs[b] @ Wv  # heads = 8, dh = 80
    probs = softmax(q k^T / sqrt(dh))                # (H, N, L), L = 77
    out = (probs @ v) @ Wo + bo                      # -> (C, H, W)
    saliency = probs.mean(heads).mean(L) = 1/L       # constant: softmax sums
                                                     # to 1 over exactly the
                                                     # axis that is averaged

Sharding: data-parallel over batch (8 elements -> 8 cores, no collectives).

On-device layout keeps every activation transposed as [channel, token] so the
contraction dim is always the partition dim and no on-device transposes are
needed (hidden_states is already (C, N) in DRAM; encoder states are transposed
on host - 77x768 is tiny). Matmuls run as float32r (fp32 data, 1 PE
cycle/row at free-dim >= 256 vs 4 for plain fp32).

Compute-engine SBUF operands must start at partition 0/32/64/96, and matmul
K > 64 must start at partition 0, so nothing may straddle head boundaries at
runtime. Instead:
  - scores contract over full 128-row channel tiles against kT fragments that
    are zero-padded outside the head's channel range (padding costs no PE
    cycles - matmul time depends only on the moving free-size). The padded
    fragments are assembled once via SBUF->SBUF DMA, which has no partition
    alignment rules.
  - v gets a ones-column per head so the attention matmul also emits the
    softmax denominator (psum row 80). Each denominator row is copied to a
    single-partition sbuf strip, broadcast back over 80 partitions with a K=1
    matmul against a ones row, reciprocated on DVE, and multiplied into the
    attention tile on GPSIMD.
  - attention output stays head-aligned ([80, n] tiles) and Wo is contracted
    head by head (K=80 from partition 0) from a host-side head-major reshape.
"""

import numpy as np

import concourse.bass as bass
import concourse.mybir as mybir
import concourse.tile as tile
from concourse import bacc
from concourse.bass_utils import run_bass_kernel_spmd

N_CORES = 8
B, C, H, W = 8, 640, 64, 64
N = H * W            # 4096 tokens per batch element
L, CD = 77, 768      # text tokens, text channels
HEADS = 8
DH = C // HEADS      # 80
SCALE = 1.0 / float(np.sqrt(DH))
TN = 512             # token tile (psum bank width in fp32)
NT = N // TN         # 8 token tiles
CT = C // 128        # 5 channel tiles
KQ = C // 128        # 5 contraction tiles for q projection
KE = CD // 128       # 6 contraction tiles for k/v projections

F32 = mybir.dt.float32
F32R = mybir.dt.float32r
AF = mybir.ActivationFunctionType


def _head_fragments(h):
    """Split head h's channel rows [h*80, (h+1)*80) by 128-partition tile.

    Yields (tile_idx, part_lo, part_hi, row_lo): SBUF tile index, partition
    range inside that tile, and the fragment's offset inside the head.
    """
    lo, hi = h * DH, (h + 1) * DH
    out = []
    while lo < hi:
        t = lo // 128
        stop = min(hi, (t + 1) * 128)
        out.append((t, lo - t * 128, stop - t * 128, lo - h * DH))
        lo = stop
    return out


FRAGS = [(h,) + f for h in range(HEADS) for f in _head_fragments(h)]
NF = len(FRAGS)  # 12


def build_bass():
    nc = bacc.Bacc(
        "TRN2", target_bir_lowering=False, debug=False, num_devices=N_CORES
    )

    hs = nc.dram_tensor("hs", [C, N], F32R, kind="ExternalInput")
    ehsT = nc.dram_tensor("ehsT", [CD, L], F32R, kind="ExternalInput")
    wq = nc.dram_tensor("wq", [C, C], F32R, kind="ExternalInput")
    wk = nc.dram_tensor("wk", [CD, C], F32R, kind="ExternalInput")
    wv = nc.dram_tensor("wv", [CD, C], F32R, kind="ExternalInput")
    wo8 = nc.dram_tensor("wo8", [DH + 1, HEADS, C], F32R, kind="ExternalInput")
    bo5 = nc.dram_tensor("bo5", [128, CT], F32, kind="ExternalInput")
    out = nc.dram_tensor("out", [C, N], F32, kind="ExternalOutput")

    hs_r = hs.ap().rearrange("(t p) n -> p t n", p=128)      # [128, 5, 4096]
    out_r = out.ap().rearrange("(t p) n -> p t n", p=128)    # [128, 5, 4096]

    with tile.TileContext(nc) as tc:
        with (
            tc.tile_pool(name="const", bufs=1) as const,
            tc.tile_pool(name="psA", bufs=4, space="PSUM") as psA,
            tc.tile_pool(name="psB", bufs=4, space="PSUM") as psB,
        ):
            # ---- setup: k/v projections (wk/ehsT freed after this pool) ----
            kTpad = const.tile([128, NF, L], F32R)
            v1_sb = const.tile([L, HEADS, DH + 1], F32R)
            nc.vector.memset(kTpad.bitcast(mybir.dt.uint32), 0)
            nc.vector.memset(v1_sb.bitcast(mybir.dt.uint32), 0x3F800000)  # 1.0f
            with tc.tile_pool(name="setup", bufs=1) as setup:
                wk_sb = setup.tile([128, KE, C], F32R)
                wv_sb = setup.tile([128, KE, C], F32R)
                ehsT_sb = setup.tile([128, KE, L + 1], F32R)
                kT8_sb = setup.tile([DH, HEADS, L], F32R)
                nc.sync.dma_start(
                    out=wk_sb, in_=wk.ap().rearrange("(t p) c -> p t c", p=128)
                )
                nc.sync.dma_start(
                    out=wv_sb, in_=wv.ap().rearrange("(t p) c -> p t c", p=128)
                )
                nc.vector.memset(ehsT_sb.bitcast(mybir.dt.uint32), 0)
                nc.sync.dma_start(
                    out=ehsT_sb[:, :, :L],
                    in_=ehsT.ap().rearrange("(t p) l -> p t l", p=128),
                )

                # kT per head: [80, L] tiles (legal base-0 evacuations) ...
                for h in range(HEADS):
                    ps_k = psB.tile([DH, L + 1], F32, tag="mmB")
                    for kt in range(KE):
                        nc.tensor.matmul(
                            ps_k,
                            lhsT=wk_sb[:, kt, h * DH:(h + 1) * DH],
                            rhs=ehsT_sb[:, kt, :],
                            start=(kt == 0),
                            stop=(kt == KE - 1),
                        )
                    nc.scalar.copy(out=kT8_sb[:, h, :], in_=ps_k[:, :L])
                # ... then zero-padded per-fragment copies of kT rows, moved
                # with DMA (no partition-alignment rules on DMA).
                for j, (h, t, p0, p1, r0) in enumerate(FRAGS):
                    nc.sync.dma_start(
                        out=kTpad[p0:p1, j, :],
                        in_=kT8_sb[r0:r0 + (p1 - p0), h, :],
                    )

                # v1 = [v | ones] per head (ones from the initial memset)
                for c0, c1 in [(0, TN), (TN, C)]:
                    ps_v = psB.tile([L, TN], F32, tag="mmB")
                    for kt in range(KE):
                        nc.tensor.matmul(
                            ps_v[:, : c1 - c0],
                            lhsT=ehsT_sb[:, kt, :L],
                            rhs=wv_sb[:, kt, c0:c1],
                            start=(kt == 0),
                            stop=(kt == KE - 1),
                        )
                    for h in range(HEADS):
                        s0, s1 = max(h * DH, c0), min((h + 1) * DH, c1)
                        if s0 < s1:
                            nc.scalar.copy(
                                out=v1_sb[:, h, 1 + s0 - h * DH:1 + s1 - h * DH],
                                in_=ps_v[:, s0 - c0:s1 - c0],
                            )

            wq_sb = const.tile([128, KQ, C], F32R)
            wo8_sb = const.tile([DH + 1, HEADS, C], F32R)
            bo_sb = const.tile([128, CT], F32)
            ones_sb = const.tile([1, DH + 1], F32R)
            nc.sync.dma_start(
                out=wq_sb, in_=wq.ap().rearrange("(t p) c -> p t c", p=128)
            )
            nc.sync.dma_start(out=wo8_sb, in_=wo8.ap())
            nc.sync.dma_start(out=bo_sb, in_=bo5.ap())
            nc.vector.memset(ones_sb.bitcast(mybir.dt.uint32), 0x3F800000)  # 1.0f

            with (
                tc.tile_pool(name="work", bufs=2) as work,
                tc.tile_pool(name="probs_p", bufs=1) as probs_p,
                tc.tile_pool(name="den_p", bufs=3) as den_p,
            ):
                for nt in range(NT):
                    nsl = slice(nt * TN, (nt + 1) * TN)

                    hs_sb = work.tile([128, KQ, TN], F32R)
                    nc.sync.dma_start(out=hs_sb, in_=hs_r[:, :, nsl])

                    # qT = Wq^T @ xT, dense [c, n] tiles
                    qT_sb = work.tile([128, CT, TN], F32R)
                    for t in range(CT):
                        ps_q = psA.tile([128, TN], F32, tag="mmA")
                        for kt in range(KQ):
                            nc.tensor.matmul(
                                ps_q,
                                lhsT=wq_sb[:, kt, t * 128:(t + 1) * 128],
                                rhs=hs_sb[:, kt, :],
                                start=(kt == 0),
                                stop=(kt == KQ - 1),
                            )
                        nc.scalar.copy(out=qT_sb[:, t, :], in_=ps_q)

                    # scores -> exp -> attention+denominator, per head
                    probs_sb = probs_p.tile([L, HEADS, TN], F32R)
                    attn_sb = work.tile([DH + 1, HEADS, TN], F32R)
                    recips = []
                    for h in range(HEADS):
                        js = [j for j, f in enumerate(FRAGS) if f[0] == h]
                        ps_s = psB.tile([L, TN], F32, tag="mmB")
                        for i, j in enumerate(js):
                            nc.tensor.matmul(
                                ps_s,
                                lhsT=kTpad[:, j, :],
                                rhs=qT_sb[:, FRAGS[j][1], :],
                                start=(i == 0),
                                stop=(i == len(js) - 1),
                            )
                        nc.scalar.activation(
                            out=probs_sb[:, h, :], in_=ps_s, func=AF.Exp,
                            scale=SCALE,
                        )
                        ps_av = psB.tile([DH + 1, TN], F32, tag="mmB")
                        nc.tensor.matmul(
                            ps_av,
                            lhsT=v1_sb[:, h, :],
                            rhs=probs_sb[:, h, :],
                            start=True,
                            stop=True,
                        )
                        # denominator row -> 1-partition strip -> K=1
                        # broadcast matmul -> reciprocal
                        den_sb = den_p.tile([1, TN], F32R, tag="den")
                        nc.vector.tensor_copy(
                            out=den_sb, in_=ps_av[0:1, :]
                        )
                        ps_bc = psA.tile([DH + 1, TN], F32, tag="mmA")
                        nc.tensor.matmul(
                            ps_bc, lhsT=ones_sb, rhs=den_sb,
                            start=True, stop=True,
                        )
                        recip_sb = den_p.tile([DH + 1, TN], F32, tag="recip")
                        nc.vector.reciprocal(out=recip_sb, in_=ps_bc)
                        nc.scalar.copy(
                            out=attn_sb[:, h, :], in_=ps_av[:, :]
                        )
                        recips.append(recip_sb)

                    # normalize on gpsimd (sbuf-only engine; DVE is busy)
                    for h in range(HEADS):
                        nc.gpsimd.tensor_mul(
                            out=attn_sb[:, h, :],
                            in0=attn_sb[:, h, :],
                            in1=recips[h],
                        )

                    # out = Wo^T @ attn + bo, head-by-head contraction
                    out_sb = work.tile([128, CT, TN], F32)
                    for t in range(CT):
                        ps_o = psA.tile([128, TN], F32, tag="mmA")
                        for h in range(HEADS):
                            nc.tensor.matmul(
                                ps_o,
                                lhsT=wo8_sb[:, h, t * 128:(t + 1) * 128],
                                rhs=attn_sb[:, h, :],
                                start=(h == 0),
                                stop=(h == HEADS - 1),
                            )
                        nc.scalar.activation(
                            out=out_sb[:, t, :], in_=ps_o, func=AF.Identity,
                            bias=bo_sb[:, t:t + 1], scale=1.0,
                        )
                        nc.sync.dma_start(
                            out=out_r[:, t, nsl], in_=out_sb[:, t, :]
                        )

    nc.compile()
    return nc


_NC_CACHE = {}


def _get_nc():
    if "nc" not in _NC_CACHE:
        _NC_CACHE["nc"] = build_bass()
    return _NC_CACHE["nc"]


def make_in_maps(hidden_states, encoder_hidden_states, Wq, Wk, Wv, Wo, bo):
    hidden_states = np.ascontiguousarray(hidden_states, dtype=np.float32)
    ehs = np.asarray(encoder_hidden_states, dtype=np.float32)
    Wq = np.ascontiguousarray(Wq, dtype=np.float32)
    Wk = np.ascontiguousarray(Wk, dtype=np.float32)
    Wv = np.ascontiguousarray(Wv, dtype=np.float32)
    # Wo rows grouped by head with a zero row prepended:
    # wo8[0, h, :] = 0 (kills the denominator row), wo8[1+d, h, c] = Wo[h*80+d, c]
    wo8 = np.zeros((DH + 1, HEADS, C), dtype=np.float32)
    wo8[1:] = np.asarray(Wo, dtype=np.float32).reshape(HEADS, DH, C).transpose(1, 0, 2)
    bo5 = np.ascontiguousarray(
        np.asarray(bo, dtype=np.float32).reshape(CT, 128).T
    )
    in_maps = []
    for b in range(B):
        in_maps.append(
            {
                "hs": hidden_states[b].reshape(C, N),
                "ehsT": np.ascontiguousarray(ehs[b].T),
                "wq": Wq,
                "wk": Wk,
                "wv": Wv,
                "wo8": wo8,
                "bo5": bo5,
            }
        )
    return in_maps


def kernel(hidden_states, encoder_hidden_states, Wq, Wk, Wv, Wo, bo):
    nc = _get_nc()
    in_maps = make_in_maps(
        hidden_states, encoder_hidden_states, Wq, Wk, Wv, Wo, bo
    )
    res = run_bass_kernel_spmd(nc, in_maps, list(range(N_CORES)))
    out = np.stack([res.results[b]["out"] for b in range(B)]).reshape(B, C, H, W)
    saliency = np.full((B, 1, H, W), 1.0 / L, dtype=np.float32)
    return out, saliency


if __name__ == "__main__":
    rng = np.random.default_rng(0)
    inputs = {
        "hidden_states": rng.standard_normal((B, C, H, W), dtype=np.float32),
        "encoder_hidden_states": rng.standard_normal((B, L, CD), dtype=np.float32),
        "Wq": rng.standard_normal((C, C), dtype=np.float32) * 0.02,
        "Wk": rng.standard_normal((CD, C), dtype=np.float32) * 0.02,
        "Wv": rng.standard_normal((CD, C), dtype=np.float32) * 0.02,
        "Wo": rng.standard_normal((C, C), dtype=np.float32) * 0.02,
        "bo": np.zeros((C,), dtype=np.float32),
    }
    out, sal = kernel(**inputs)
    print("out", out.shape, "sal", sal.shape)
